# revision 22
# baseline (speedup 1.0000x reference)
"""Fused QKV-projection + multi-head attention kernel for Trainium2.

Problem: x [2, 2048, 1024] fp32; W_qkv [1024, 3072]; b_qkv [3072].
  qkv = x @ W_qkv + b; 16 heads of 64; scores = q k^T / 8; softmax; out = attn @ v.

Sharding: 8 cores = 2 (batch) x 4 (head groups of 4). Each core is fully
independent (no collectives): projection for its batch restricted to its 4
heads' q/k/v columns, then attention for those heads.

Per-core design:
  - host feeds x^T with an appended ones-row (feeds V's bias fold);
    q/k biases are applied on DVE during the PSUM->SBUF copy
    (tensor_scalar_add with a per-partition [128,1] bias column), so the
    q/k projection contracts over exactly 1024 rows = 8 full PE chunks.
  - q/k are produced TRANSPOSED and packed in head-PAIR tiles [128, S]
    (rows 0-63 = even head, 64-127 = odd head). The two halves drive two
    matmuls on disjoint PE row-groups (tile_position auto-derived from the
    base partition) that execute CONCURRENTLY on the 128x128 array --
    recovering the half-array loss of the Dh=64 contraction.
  - scores^T = kT.T @ qT needs no transposes anywhere; softmax is a single
    fused exp on ScalarE (scale=1/8 applied by the ACT datapath; no max
    subtraction -- scores are O(+-8), well within fp16/fp32 exp range).
  - V tiles are [128 k, 4 heads x 65]: per head 64 projected v-cols plus a
    ones column generated by the bias row (x's appended ones-row), which
    rides the PV matmul to produce the softmax denominators.
  - PV runs FLIPPED: out[q, v] = expS.T @ V_aug with the EXP TILE as the
    matmul stationary ([128 k, 128 q] f16, 128 cols => compiler-automatic
    fast weight load; measured 40.8 ns/matmul steady state) and V_aug
    [128 k, 65] as the stream: 65 streamed columns per 128 q-positions,
    ~2.3x fewer PE stream cycles than streaming exp past a V stationary.
    Each (head, q-chunk) accumulator chain closes its PSUM group before
    the next opens in that bank (one pending group per psum zero-region).
    The denominator lands as a per-PARTITION column, so normalization is a
    [128,1] reciprocal + per-partition-scalar multiply -- sub-microsecond,
    with no single-partition row ops, no DRAM bounce, and a tiny tail.
    The exp is pre-scaled by e^-9 (free ACT bias, cancels exactly in the
    softmax ratio) so denominators sit comfortably in fp16.
  - the output is stored in natural [seq, head-dim] orientation, fp16
    (host upcasts).
  - matmul operands are fp16 (measured end-to-end rel err ~1.4e-3 vs the fp32
    reference; strict-fp32 matmuls are 4x slower, fp32r trips walrus sync
    limits). PSUM accumulation is fp32.

Scheduling: Tile tracks dependencies in EMISSION order, while scheduler
priority is tc.cur_priority -- V production is emitted early (correct deps)
but in a low-priority band so it fills PE slack behind the ACT-bound exp
stream; the pair-1 projection overlaps pair-0's attention the same way.
Emission order is arranged so pair-0 attention's prerequisites (one q
block + all of kP[0] + V) are produced first.

Sync: this walrus build rejects instructions carrying more than one embedded
semaphore wait. _relax_waits() strips provably redundant waits (PE self-waits;
same-engine-covered waits) and _split_multi_waits() hoists any remaining
excess onto single-wait NoOps inserted before the instruction.
"""

import sys

if "/opt/trn_rl_repo" not in sys.path:
    sys.path.insert(0, "/opt/trn_rl_repo")

import numpy as np
from contextlib import ExitStack

B, S, D = 2, 2048, 1024
H, Dh = 16, 64
HL = 4          # heads per core
GW = HL * Dh    # 256 output cols per core
VW = HL * 65    # V width: per head [v (64) | ones (1)] -- ones col via bias row
KC = 1025       # augmented contraction for V (1024 + bias row)
NST = S // 128  # 16 s-tiles
NQB = S // 512  # 4 q blocks

_CACHE = {}


def _build_nc():
    import concourse.bass as bass
    import concourse.mybir as mybir
    import concourse.tile as tile

    f32 = mybir.dt.float32
    f16 = mybir.dt.float16
    bf16 = mybir.dt.bfloat16
    Exp = mybir.ActivationFunctionType.Exp

    nc = bass.Bass()
    xT = nc.dram_tensor("xT", [KC, S], f16, kind="ExternalInput")
    wqk = nc.dram_tensor("wqk", [D, 512], f16, kind="ExternalInput")
    wv = nc.dram_tensor("wv", [KC, VW], f16, kind="ExternalInput")
    bqk = nc.dram_tensor("bqk", [128, 4], f32, kind="ExternalInput")
    # output is stored in natural [seq, head-dim] orientation, fp16 (host
    # upcasts): the PV matmuls run FLIPPED -- stationary = exp tile slice
    # [128 k, 128 q] (f16, 128-col => compiler-automatic fast weight load:
    # measured 40.8 ns/matmul steady state), stream = V_aug [128 k, 65]
    # (64 v-cols + ones col). 65 streamed columns per 128 q-positions is
    # ~2.3x fewer PE stream cycles than the [v,q] orientation, and the
    # softmax denominator lands as a per-PARTITION column [128,1], making
    # normalization two tiny DVE ops (reciprocal [128,1] + tensor_scalar
    # multiply). The exp is emitted pre-scaled by e^-9 (free ACT bias,
    # cancels exactly in the softmax ratio) so denominators sit in fp16
    # range.
    out = nc.dram_tensor("out", [S, GW], f16, kind="ExternalOutput")

    def chunks(n=8):
        for d in range(n):
            yield d, 128

    def vchunks():
        for d in range(9):
            yield d, (128 if d < 8 else 1)

    with tile.TileContext(nc) as tc, ExitStack() as ctx:
        persist = ctx.enter_context(tc.tile_pool(name="persist", bufs=1))
        # q/k head-PAIR tiles: rows 0..63 = head 2p, 64..127 = head 2p+1.
        qP = [persist.tile([128, S], f16, name=f"qP{p}", tag=f"qP{p}") for p in range(2)]
        kP = [persist.tile([128, S], f16, name=f"kP{p}", tag=f"kP{p}") for p in range(2)]
        V = [persist.tile([128, VW], f16, name=f"V{t}", tag=f"V{t}") for t in range(NST)]
        bqk_sb = persist.tile([128, 4], f32, name="bqk_sb", tag="bqk_sb")
        # exp pre-shift constant (see `out` comment): exp(s/8 - 9)
        nbias = persist.tile([128, 1], f32, name="nbias", tag="nbias")
        nc.vector.memset(nbias, -9.0)

        wpool = ctx.enter_context(tc.tile_pool(name="wpool", bufs=1))
        xpool = ctx.enter_context(tc.tile_pool(name="xpool", bufs=1))

        # input DMAs, ordered so pair-0 attention prerequisites land first.
        # Weights and x are split into SEPARATE tiles at the granularity the
        # first projection chain consumes (per-tile dependency tracking), so
        # the first matmul waits on ~160KB, not the whole input set:
        #   wqA [128,128] = q-pair-0 weight block, wqB [128,384] = the rest;
        #   xA/xB = 512-col halves of each 1024-col x chunk.
        nc.sync.dma_start(out=bqk_sb, in_=bqk[:, :])
        wqA, wqB, wv_sb = [], [], []
        xA = [[], []]
        xB = [[], []]
        for d, p in chunks():
            twq = wpool.tile([p, 128], f16, name=f"wqA{d}", tag=f"wqA{d}")
            nc.sync.dma_start(out=twq, in_=wqk[d * 128:d * 128 + p, 0:128])
            wqA.append(twq)
            t0 = xpool.tile([p, 512], f16, name=f"xA0_{d}", tag=f"xA0_{d}")
            nc.sync.dma_start(out=t0, in_=xT[d * 128:d * 128 + p, 0:512])
            xA[0].append(t0)
        for d, p in chunks():
            twq = wpool.tile([p, 384], f16, name=f"wqB{d}", tag=f"wqB{d}")
            nc.sync.dma_start(out=twq, in_=wqk[d * 128:d * 128 + p, 128:512])
            wqB.append(twq)
            t0 = xpool.tile([p, 512], f16, name=f"xB0_{d}", tag=f"xB0_{d}")
            nc.sync.dma_start(out=t0, in_=xT[d * 128:d * 128 + p, 512:1024])
            xB[0].append(t0)
        for d, p in chunks():
            t1 = xpool.tile([p, 512], f16, name=f"xA1_{d}", tag=f"xA1_{d}")
            nc.sync.dma_start(out=t1, in_=xT[d * 128:d * 128 + p, 1024:1536])
            xA[1].append(t1)
            t1 = xpool.tile([p, 512], f16, name=f"xB1_{d}", tag=f"xB1_{d}")
            nc.sync.dma_start(out=t1, in_=xT[d * 128:d * 128 + p, 1536:2048])
            xB[1].append(t1)
        # V inputs: 9th chunk of x (the ones row, feeds V's bias fold)
        x8 = []
        for d, p in vchunks():
            twv = wpool.tile([p, VW], f16, name=f"wv{d}", tag=f"wv{d}")
            nc.sync.dma_start(out=twv, in_=wv[d * 128:d * 128 + p, :])
            wv_sb.append(twv)
            if d == 8:
                for sh in range(2):
                    t8 = xpool.tile([p, 1024], f16, name=f"x8_{sh}", tag=f"x8_{sh}")
                    nc.sync.dma_start(
                        out=t8, in_=xT[1024:1025, sh * 1024:(sh + 1) * 1024])
                    x8.append(t8)

        def xstream(sh, j, d):
            """x operand [*,512] for q-block j of seq-half sh, chunk d."""
            return (xA if j == 0 else xB)[sh][d]

        def wq_block(mt, d):
            """weight stationary [*,128] for projection M-tile mt, chunk d."""
            return wqA[d] if mt == 0 else wqB[d][:, (mt - 1) * 128:mt * 128]

        with tc.tile_pool(name="psA", bufs=2, space="PSUM") as psA, \
             tc.tile_pool(name="expp", bufs=40) as expp, \
             tc.tile_pool(name="normp", bufs=6) as normp, \
             tc.tile_pool(name="psS", bufs=2, space="PSUM") as psS, \
             tc.tile_pool(name="psO", bufs=2, space="PSUM") as psO:

            def qk_half(sh, mt, j):
                """Half a projection M-tile (one 512-col q-block) -> qP/kP.
                mt 0/1 -> q pairs 0/1, mt 2/3 -> k pairs 0/1. Bias applied
                on DVE during the PSUM->SBUF copy (bqk col: q pairs 0/1 ->
                cols 0/1, k pairs 0/1 -> cols 2/3)."""
                dst = (qP if mt < 2 else kP)[mt % 2]
                ps = psA.tile([128, 512], f32, name=f"psA{sh}_{mt}_{j}", tag="psA")
                for d, p in chunks():
                    nc.tensor.matmul(ps, wq_block(mt, d), xstream(sh, j, d),
                                     start=(d == 0), stop=(d == 7))
                qb = sh * 2 + j
                bcol = (mt % 2) if mt < 2 else (2 + mt % 2)
                nc.vector.tensor_scalar_add(
                    dst[:, qb * 512:(qb + 1) * 512], ps, bqk_sb[:, bcol:bcol + 1])

            def qk_group(sh, mt):
                qk_half(sh, mt, 0)
                qk_half(sh, mt, 1)

            def v_group(st):
                sh, stl = divmod(st, 8)
                psv = psA.tile([128, VW], f32, name=f"psV{st}", tag="psA")
                for d, p in vchunks():
                    if d < 8:
                        xop = (xA if stl < 4 else xB)[sh][d][
                            :, (stl % 4) * 128:(stl % 4) * 128 + 128]
                    else:
                        xop = x8[sh][:, stl * 128:(stl + 1) * 128]
                    nc.tensor.matmul(psv, xop, wv_sb[d],
                                     start=(d == 0), stop=(d == 8))
                nc.vector.tensor_copy(V[st], psv)

            def attention_iter(p, qb):
                # packed scores^T: head 2p on PE rows 0-63 -> psS bank 0,
                # head 2p+1 on rows 64-127 -> bank 1; one fused exp over both
                ets = []
                for st in range(NST):
                    ps = psS.tile([128, 1024], f32, name=f"s{p}_{qb}_{st}", tag="psS")
                    for hh in range(2):
                        r0, r1 = hh * 64, hh * 64 + 64
                        nc.tensor.matmul(
                            ps[:, hh * 512:(hh + 1) * 512],
                            kP[p][r0:r1, st * 128:(st + 1) * 128],
                            qP[p][r0:r1, qb * 512:(qb + 1) * 512],
                            start=True, stop=True)
                    et = expp.tile([128, 1024], f16, name=f"e{p}_{qb}_{st}", tag="expS")
                    nc.scalar.activation(et, ps, Exp, scale=0.125, bias=nbias[:, 0:1])
                    ets.append(et)

                # FLIPPED PV: out[q, v] = exp(S^T).T @ V_aug, accumulated
                # over st. Stationary = exp slice [128 k, 128 q] (fast
                # weight load), stream = V_aug [128 k, 65]. Each head gets
                # 4 q-chunk accumulator chains packed at 65-col offsets in
                # one PSUM tile; col 64 of each chunk = the softmax
                # denominator as a per-partition column.
                po = [psO.tile([128, 260], f32, name=f"po{p}_{qb}_{hh}",
                               tag="psO") for hh in range(2)]
                # chain-major: each (head, q-chunk) chain closes its psum
                # accumulation group before the next opens in that bank
                # (one pending group per psum zero-region). PV trails the
                # exp stream by up to one iteration; expp is deep enough.
                for hh in range(2):
                    h = 2 * p + hh
                    for c in range(4):
                        for st in range(NST):
                            nc.tensor.matmul(
                                po[hh][:, c * 65:(c + 1) * 65],
                                ets[st][:, hh * 512 + c * 128:hh * 512 + (c + 1) * 128],
                                V[st][:, h * 65:(h + 1) * 65],
                                start=(st == 0), stop=(st == NST - 1))
                # normalize: per (head, q-chunk) a [128,1] reciprocal and a
                # per-partition-scalar multiply -- tiny full-width DVE ops
                for hh in range(2):
                    h = 2 * p + hh
                    for c in range(4):
                        rec = normp.tile([128, 1], f32, name=f"rc{p}_{qb}_{hh}_{c}",
                                         tag="rec")
                        nc.vector.reciprocal(rec, po[hh][:, c * 65 + 64:c * 65 + 65])
                        osb = normp.tile([128, 64], f16, name=f"ob{p}_{qb}_{hh}_{c}",
                                         tag="osb")
                        nc.vector.tensor_scalar_mul(osb, po[hh][:, c * 65:c * 65 + 64],
                                                    rec)
                        nc.sync.dma_start(
                            out=out[qb * 512 + c * 128:qb * 512 + (c + 1) * 128,
                                    h * 64:(h + 1) * 64],
                            in_=osb)

            # Dependency tracking is emission-order based: every producer must
            # be emitted before its consumers. Scheduling PRIORITY, however, is
            # tc.cur_priority, which we can band-shift: V is emitted early (so
            # PV sees its writes) but in a low-priority band, making it PE
            # slack-filler behind the ACT-feeding scores stream.
            # Emission order: minimal prerequisites of attention (0, qb=0)
            # first -- one q block + all of kP[0] -- then V, then the rest.
            qk_half(0, 0, 0)            # qP[0] block qb=0
            qk_group(0, 2)              # kP[0] first half of S
            qk_group(1, 2)              # kP[0] second half of S
            p_save = tc.cur_priority
            tc.cur_priority = p_save + 600
            for st in range(NST):
                v_group(st)
            tc.cur_priority += 600
            qk_half(0, 0, 1)            # qP[0] qb=1
            qk_group(1, 0)              # qP[0] qb=2,3
            qk_group(0, 3)              # kP[1]
            qk_group(1, 3)
            qk_group(0, 1)              # qP[1]
            qk_group(1, 1)
            p_proj_end = tc.cur_priority
            tc.cur_priority = p_save
            for qb in range(NQB):
                attention_iter(0, qb)
            tc.cur_priority = max(tc.cur_priority, p_proj_end)
            for qb in range(NQB):
                attention_iter(1, qb)
    return nc


def _relax_waits(nc):
    """Walrus rejects instructions carrying more than ~1 embedded semaphore
    wait ("Too many sync wait commands"). Strip waits that are provably
    redundant. Soundness (this kernel is fully unrolled: no loops, no sem
    resets, all sems monotone):
      R1: a PE instruction never needs a wait on PE's own completion
          semaphore: PE executes in order, never reads its own output
          (no PSUM read port), and drains (PSUM writes) are in order.
      R2: a wait (sem >= v) is redundant if an earlier instruction on the
          same engine already waits (sem >= v' >= v): the per-engine
          sequencer processes waits in stream order.
    Returns the number of instructions still carrying >1 ge-waits."""
    # Only PE: it never reads its own writes (no PSUM read port), and its
    # in-order drain sequences PSUM WAW. DVE/ACT have deep non-interlocked
    # pipelines -- their self-waits guard real RAW hazards.
    own_sem = {"PE": "PE_"}
    observed = {}  # (engine, sem id) -> max value waited
    remaining = 0
    for fn in nc.m.functions:
        for blk in fn.blocks:
            for inst in blk.instructions:
                si = getattr(inst, "sync_info", None)
                if si is None or not si.on_wait:
                    continue
                eng = str(inst.engine).split(".")[-1]
                pfx = own_sem.get(eng)
                keep, nge = [], 0
                for w in si.on_wait:
                    if w.sync_type != "semaphore" or w.wait_mode != "sem-ge-imm" \
                            or w.wait_reg is not None \
                            or w.ant_name.startswith("barrier_"):
                        # barrier sems are decremented (non-monotone): hands off
                        keep.append(w)
                        continue
                    if pfx is not None and w.ant_name.startswith(pfx):
                        continue  # R1
                    k = (eng, w.id)
                    if observed.get(k, -1) >= w.wait_value:
                        continue  # R2
                    observed[k] = w.wait_value
                    keep.append(w)
                    nge += 1
                if nge > 1:
                    remaining += 1
                if len(keep) != len(si.on_wait):
                    si.on_wait = keep
                    inst.sync_info = si
    return remaining


def _split_multi_waits(nc):
    """Any instruction still carrying >1 ge-waits after relaxation gets its
    excess waits hoisted onto same-engine NoOps inserted right before it
    (a sequence of single-wait instructions is semantically identical to one
    multi-wait instruction on an in-order sequencer)."""
    import bass_rust

    def wkey(w):
        return (w.id, w.wait_value, w.wait_mode)

    plan = {}
    for fn in nc.m.functions:
        for blk in fn.blocks:
            for inst in blk.instructions:
                si = getattr(inst, "sync_info", None)
                if si is None or not si.on_wait:
                    continue
                ow = list(si.on_wait)
                ge = [w for w in ow
                      if w.sync_type == "semaphore" and w.wait_mode == "sem-ge-imm"
                      and w.wait_reg is None
                      and not w.ant_name.startswith("barrier_")]
                if len(ge) <= 1:
                    continue
                hoist = ge[1:]
                hkeys = {wkey(w) for w in hoist}
                nops = []
                for w in hoist:
                    nb = nc.engines[inst.engine].nop(nofuse=True, hint="wait_split")
                    ni = nb.ins
                    ni.sync_info = bass_rust.SyncInfo(on_wait=[w], on_update=[])
                    nops.append(ni)
                plan[inst.name] = nops
                si.on_wait = [w for w in ow if wkey(w) not in hkeys
                              or (w.sync_type, w.wait_mode) != ("semaphore", "sem-ge-imm")]
                inst.sync_info = si
    if not plan:
        return 0
    created = {n.name for nops in plan.values() for n in nops}
    for fn in nc.m.functions:
        for blk in fn.blocks:
            cur = list(blk.instructions)
            new = []
            for i in cur:
                if i.name in created:
                    continue
                if i.name in plan:
                    new.extend(plan[i.name])
                new.append(i)
            blk.instructions = new
    return len(plan)


def get_nc():
    if "nc" not in _CACHE:
        nc = _build_nc()
        _relax_waits(nc)
        _split_multi_waits(nc)
        _CACHE["nc"] = nc
    return _CACHE["nc"]


def prep_inputs(x, W_qkv, b_qkv):
    """Host-side sharding: returns the 8 per-core input maps."""
    x = np.asarray(x, dtype=np.float32)
    W_qkv = np.asarray(W_qkv, dtype=np.float32)
    b_qkv = np.asarray(b_qkv, dtype=np.float32)
    ones = np.ones((1, S), np.float32)
    in_maps = []
    for c in range(8):
        b, g = divmod(c, 4)
        xTm = np.concatenate([np.ascontiguousarray(x[b].T), ones], axis=0).astype(np.float16)
        heads = list(range(HL * g, HL * g + HL))
        cols = np.concatenate([np.arange(h * Dh, (h + 1) * Dh) for h in heads])
        wqk_m = np.empty((D, 512), np.float16)
        wqk_m[:, :256] = W_qkv[:, cols]
        wqk_m[:, 256:] = W_qkv[:, D + cols]
        # bias columns: [q pair0 | q pair1 | k pair0 | k pair1]
        bqk_m = np.empty((128, 4), np.float32)
        bqk_m[:, 0] = b_qkv[cols[:128]]
        bqk_m[:, 1] = b_qkv[cols[128:]]
        bqk_m[:, 2] = b_qkv[D + cols[:128]]
        bqk_m[:, 3] = b_qkv[D + cols[128:]]
        wv_m = np.zeros((KC, VW), np.float16)
        for i, h in enumerate(heads):
            vcols = 2 * D + h * Dh
            wv_m[:D, i * 65:i * 65 + 64] = W_qkv[:, vcols:vcols + Dh]
            wv_m[D, i * 65:i * 65 + 64] = b_qkv[vcols:vcols + Dh]
            wv_m[D, i * 65 + 64] = 1.0  # generates the denominator column
        in_maps.append({"xT": xTm, "wqk": wqk_m, "wv": wv_m, "bqk": bqk_m})
    return in_maps


def assemble_output(results):
    out = np.empty((B, S, D), np.float32)
    for c in range(8):
        b, g = divmod(c, 4)
        out[b, :, g * GW:(g + 1) * GW] = np.asarray(results[c]["out"]).astype(np.float32)
    return out


def kernel(x, W_qkv, b_qkv):
    from concourse.bass_utils import run_bass_kernel_spmd

    nc = get_nc()
    in_maps = prep_inputs(x, W_qkv, b_qkv)
    res = run_bass_kernel_spmd(nc, in_maps, list(range(8)))
    return assemble_output(res.results)


# revision 23
# speedup vs baseline: 1.0824x; 1.0824x over previous
"""Fused QKV-projection + multi-head attention kernel for Trainium2.

Problem: x [2, 2048, 1024] fp32; W_qkv [1024, 3072]; b_qkv [3072].
  qkv = x @ W_qkv + b; 16 heads of 64; scores = q k^T / 8; softmax; out = attn @ v.

Sharding: 8 cores = 2 (batch) x 4 (head groups of 4). Each core is fully
independent (no collectives): projection for its batch restricted to its 4
heads' q/k/v columns, then attention for those heads.

Per-core design:
  - host feeds x^T with an appended ones-row (feeds V's bias fold);
    q/k biases are applied on DVE during the PSUM->SBUF copy
    (tensor_scalar_add with a per-partition [128,1] bias column), so the
    q/k projection contracts over exactly 1024 rows = 8 full PE chunks.
  - q/k are produced TRANSPOSED and packed in head-PAIR tiles [128, S]
    (rows 0-63 = even head, 64-127 = odd head). The two halves drive two
    matmuls on disjoint PE row-groups (tile_position auto-derived from the
    base partition) that execute CONCURRENTLY on the 128x128 array --
    recovering the half-array loss of the Dh=64 contraction.
  - scores^T = kT.T @ qT needs no transposes anywhere; softmax is a single
    fused exp on ScalarE (scale=1/8 applied by the ACT datapath; no max
    subtraction -- scores are O(+-8), well within fp16/fp32 exp range).
  - V tiles are [128 k, 4 heads x 65]: per head 64 projected v-cols plus a
    ones column generated by the bias row (x's appended ones-row), which
    rides the PV matmul to produce the softmax denominators.
  - PV runs FLIPPED: out[q, v] = expS.T @ V_aug with the EXP TILE as the
    matmul stationary ([128 k, 128 q] f16, 128 cols => compiler-automatic
    fast weight load; measured 40.8 ns/matmul steady state) and V_aug
    [128 k, 65] as the stream: 65 streamed columns per 128 q-positions,
    ~2.3x fewer PE stream cycles than streaming exp past a V stationary.
    Each (head, q-chunk) accumulator chain closes its PSUM group before
    the next opens in that bank (one pending group per psum zero-region).
    The denominator lands as a per-PARTITION column, so normalization is a
    [128,1] reciprocal + per-partition-scalar multiply -- sub-microsecond,
    with no single-partition row ops, no DRAM bounce, and a tiny tail.
    The exp is pre-scaled by e^-9 (free ACT bias, cancels exactly in the
    softmax ratio) so denominators sit comfortably in fp16.
  - the output is stored in natural [seq, head-dim] orientation, fp16
    (host upcasts).
  - matmul operands are fp16 (measured end-to-end rel err ~1.4e-3 vs the fp32
    reference; strict-fp32 matmuls are 4x slower, fp32r trips walrus sync
    limits). PSUM accumulation is fp32.

Scheduling: Tile tracks dependencies in EMISSION order, while scheduler
priority is tc.cur_priority -- V production is emitted early (correct deps)
but in a low-priority band so it fills PE slack behind the ACT-bound exp
stream; the pair-1 projection overlaps pair-0's attention the same way.
Emission order is arranged so pair-0 attention's prerequisites (one q
block + all of kP[0] + V) are produced first.

Sync: this walrus build rejects instructions carrying more than one embedded
semaphore wait. _relax_waits() strips provably redundant waits (PE self-waits;
same-engine-covered waits) and _split_multi_waits() hoists any remaining
excess onto single-wait NoOps inserted before the instruction.
"""

import sys

if "/opt/trn_rl_repo" not in sys.path:
    sys.path.insert(0, "/opt/trn_rl_repo")

import numpy as np
from contextlib import ExitStack

B, S, D = 2, 2048, 1024
H, Dh = 16, 64
HL = 4          # heads per core
GW = HL * Dh    # 256 output cols per core
VW = HL * 65    # V width: per head [v (64) | ones (1)] -- ones col via bias row
KC = 1025       # augmented contraction for V (1024 + bias row)
NST = S // 128  # 16 s-tiles
NQB = S // 512  # 4 q blocks

_CACHE = {}


def _build_nc():
    import concourse.bass as bass
    import concourse.mybir as mybir
    import concourse.tile as tile

    f32 = mybir.dt.float32
    f16 = mybir.dt.float16
    bf16 = mybir.dt.bfloat16
    Exp = mybir.ActivationFunctionType.Exp

    nc = bass.Bass()
    xT = nc.dram_tensor("xT", [KC, S], f16, kind="ExternalInput")
    wqk = nc.dram_tensor("wqk", [D, 512], f16, kind="ExternalInput")
    wv = nc.dram_tensor("wv", [KC, VW], f16, kind="ExternalInput")
    bqk = nc.dram_tensor("bqk", [128, 4], f32, kind="ExternalInput")
    # output is stored in natural [seq, head-dim] orientation, fp16 (host
    # upcasts): the PV matmuls run FLIPPED -- stationary = exp tile slice
    # [128 k, 128 q] (f16, 128-col => compiler-automatic fast weight load:
    # measured 40.8 ns/matmul steady state), stream = V_aug [128 k, 65]
    # (64 v-cols + ones col). 65 streamed columns per 128 q-positions is
    # ~2.3x fewer PE stream cycles than the [v,q] orientation, and the
    # softmax denominator lands as a per-PARTITION column [128,1], making
    # normalization two tiny DVE ops (reciprocal [128,1] + tensor_scalar
    # multiply). The exp is emitted pre-scaled by e^-9 (free ACT bias,
    # cancels exactly in the softmax ratio) so denominators sit in fp16
    # range.
    out = nc.dram_tensor("out", [S, GW], f16, kind="ExternalOutput")

    def chunks(n=8):
        for d in range(n):
            yield d, 128

    def vchunks():
        for d in range(9):
            yield d, (128 if d < 8 else 1)

    with tile.TileContext(nc) as tc, ExitStack() as ctx:
        persist = ctx.enter_context(tc.tile_pool(name="persist", bufs=1))
        # q/k head-PAIR tiles: rows 0..63 = head 2p, 64..127 = head 2p+1.
        qP = [persist.tile([128, S], f16, name=f"qP{p}", tag=f"qP{p}") for p in range(2)]
        kP = [persist.tile([128, S], f16, name=f"kP{p}", tag=f"kP{p}") for p in range(2)]
        V = [persist.tile([128, VW], f16, name=f"V{t}", tag=f"V{t}") for t in range(NST)]
        bqk_sb = persist.tile([128, 4], f32, name="bqk_sb", tag="bqk_sb")
        # exp pre-shift constant (see `out` comment): exp(s/8 - 9)
        nbias = persist.tile([128, 1], f32, name="nbias", tag="nbias")
        nc.vector.memset(nbias, -9.0)

        wpool = ctx.enter_context(tc.tile_pool(name="wpool", bufs=1))
        xpool = ctx.enter_context(tc.tile_pool(name="xpool", bufs=1))

        # input DMAs, ordered so pair-0 attention prerequisites land first.
        # Weights and x are split into SEPARATE tiles at the granularity the
        # first projection chain consumes (per-tile dependency tracking), so
        # the first matmul waits on ~160KB, not the whole input set:
        #   wqA [128,128] = q-pair-0 weight block, wqB [128,384] = the rest;
        #   xA/xB = 512-col halves of each 1024-col x chunk.
        nc.sync.dma_start(out=bqk_sb, in_=bqk[:, :])
        wqA, wqB, wv_sb = [], [], []
        xA = [[], []]
        xB = [[], []]
        for d, p in chunks():
            twq = wpool.tile([p, 128], f16, name=f"wqA{d}", tag=f"wqA{d}")
            nc.sync.dma_start(out=twq, in_=wqk[d * 128:d * 128 + p, 0:128])
            wqA.append(twq)
            t0 = xpool.tile([p, 512], f16, name=f"xA0_{d}", tag=f"xA0_{d}")
            nc.sync.dma_start(out=t0, in_=xT[d * 128:d * 128 + p, 0:512])
            xA[0].append(t0)
        for d, p in chunks():
            twq = wpool.tile([p, 384], f16, name=f"wqB{d}", tag=f"wqB{d}")
            nc.sync.dma_start(out=twq, in_=wqk[d * 128:d * 128 + p, 128:512])
            wqB.append(twq)
            t0 = xpool.tile([p, 512], f16, name=f"xB0_{d}", tag=f"xB0_{d}")
            nc.sync.dma_start(out=t0, in_=xT[d * 128:d * 128 + p, 512:1024])
            xB[0].append(t0)
        for d, p in chunks():
            t1 = xpool.tile([p, 512], f16, name=f"xA1_{d}", tag=f"xA1_{d}")
            nc.sync.dma_start(out=t1, in_=xT[d * 128:d * 128 + p, 1024:1536])
            xA[1].append(t1)
            t1 = xpool.tile([p, 512], f16, name=f"xB1_{d}", tag=f"xB1_{d}")
            nc.sync.dma_start(out=t1, in_=xT[d * 128:d * 128 + p, 1536:2048])
            xB[1].append(t1)
        # V inputs: 9th chunk of x (the ones row, feeds V's bias fold)
        x8 = []
        for d, p in vchunks():
            twv = wpool.tile([p, VW], f16, name=f"wv{d}", tag=f"wv{d}")
            nc.sync.dma_start(out=twv, in_=wv[d * 128:d * 128 + p, :])
            wv_sb.append(twv)
            if d == 8:
                for sh in range(2):
                    t8 = xpool.tile([p, 1024], f16, name=f"x8_{sh}", tag=f"x8_{sh}")
                    nc.sync.dma_start(
                        out=t8, in_=xT[1024:1025, sh * 1024:(sh + 1) * 1024])
                    x8.append(t8)

        def xstream(sh, j, d):
            """x operand [*,512] for q-block j of seq-half sh, chunk d."""
            return (xA if j == 0 else xB)[sh][d]

        def wq_block(mt, d):
            """weight stationary [*,128] for projection M-tile mt, chunk d."""
            return wqA[d] if mt == 0 else wqB[d][:, (mt - 1) * 128:mt * 128]

        with tc.tile_pool(name="psA", bufs=2, space="PSUM") as psA, \
             tc.tile_pool(name="expp", bufs=40) as expp, \
             tc.tile_pool(name="normp", bufs=6) as normp, \
             tc.tile_pool(name="psS", bufs=2, space="PSUM") as psS, \
             tc.tile_pool(name="psO", bufs=2, space="PSUM") as psO:

            def qk_half(sh, mt, j):
                """Half a projection M-tile (one 512-col q-block) -> qP/kP.
                mt 0/1 -> q pairs 0/1, mt 2/3 -> k pairs 0/1. Bias applied
                on DVE during the PSUM->SBUF copy (bqk col: q pairs 0/1 ->
                cols 0/1, k pairs 0/1 -> cols 2/3)."""
                dst = (qP if mt < 2 else kP)[mt % 2]
                ps = psA.tile([128, 512], f32, name=f"psA{sh}_{mt}_{j}", tag="psA")
                for d, p in chunks():
                    nc.tensor.matmul(ps, wq_block(mt, d), xstream(sh, j, d),
                                     start=(d == 0), stop=(d == 7))
                qb = sh * 2 + j
                bcol = (mt % 2) if mt < 2 else (2 + mt % 2)
                nc.vector.tensor_scalar_add(
                    dst[:, qb * 512:(qb + 1) * 512], ps, bqk_sb[:, bcol:bcol + 1])

            def qk_group(sh, mt):
                qk_half(sh, mt, 0)
                qk_half(sh, mt, 1)

            def v_group(st):
                sh, stl = divmod(st, 8)
                psv = psA.tile([128, VW], f32, name=f"psV{st}", tag="psA")
                for d, p in vchunks():
                    if d < 8:
                        xop = (xA if stl < 4 else xB)[sh][d][
                            :, (stl % 4) * 128:(stl % 4) * 128 + 128]
                    else:
                        xop = x8[sh][:, stl * 128:(stl + 1) * 128]
                    nc.tensor.matmul(psv, xop, wv_sb[d],
                                     start=(d == 0), stop=(d == 8))
                nc.vector.tensor_copy(V[st], psv)

            def attention_iter(p, qb):
                # packed scores^T: head 2p on PE rows 0-63 -> psS bank 0,
                # head 2p+1 on rows 64-127 -> bank 1; one fused exp over both
                ets = []
                for st in range(NST):
                    ps = psS.tile([128, 1024], f32, name=f"s{p}_{qb}_{st}", tag="psS")
                    for hh in range(2):
                        r0, r1 = hh * 64, hh * 64 + 64
                        nc.tensor.matmul(
                            ps[:, hh * 512:(hh + 1) * 512],
                            kP[p][r0:r1, st * 128:(st + 1) * 128],
                            qP[p][r0:r1, qb * 512:(qb + 1) * 512],
                            start=True, stop=True)
                    et = expp.tile([128, 1024], f16, name=f"e{p}_{qb}_{st}", tag="expS")
                    nc.scalar.activation(et, ps, Exp, scale=0.125, bias=nbias[:, 0:1])
                    ets.append(et)

                # FLIPPED PV: out[q, v] = exp(S^T).T @ V_aug, accumulated
                # over st. Stationary = exp slice [128 k, 128 q] (fast
                # weight load), stream = V_aug [128 k, 65]. Each head gets
                # 4 q-chunk accumulator chains packed at 65-col offsets in
                # one PSUM tile; col 64 of each chunk = the softmax
                # denominator as a per-partition column.
                po = [psO.tile([128, 260], f32, name=f"po{p}_{qb}_{hh}",
                               tag="psO") for hh in range(2)]
                # chain-major: each (head, q-chunk) chain closes its psum
                # accumulation group before the next opens in that bank
                # (one pending group per psum zero-region). PV trails the
                # exp stream by up to one iteration; expp is deep enough.
                # PV runs in a band below scores (the next iteration's
                # scores must preempt it -- ACT is the pacer and otherwise
                # starves ~5us at every iteration boundary) but above the
                # late projection band (expp must keep draining).
                pv_pri = tc.cur_priority
                tc.cur_priority = pv_pri + 1200
                for hh in range(2):
                    h = 2 * p + hh
                    for c in range(4):
                        for st in range(NST):
                            nc.tensor.matmul(
                                po[hh][:, c * 65:(c + 1) * 65],
                                ets[st][:, hh * 512 + c * 128:hh * 512 + (c + 1) * 128],
                                V[st][:, h * 65:(h + 1) * 65],
                                start=(st == 0), stop=(st == NST - 1))
                tc.cur_priority = pv_pri
                # normalize: per (head, q-chunk) a [128,1] reciprocal and a
                # per-partition-scalar multiply -- tiny full-width DVE ops
                for hh in range(2):
                    h = 2 * p + hh
                    for c in range(4):
                        rec = normp.tile([128, 1], f32, name=f"rc{p}_{qb}_{hh}_{c}",
                                         tag="rec")
                        nc.vector.reciprocal(rec, po[hh][:, c * 65 + 64:c * 65 + 65])
                        osb = normp.tile([128, 64], f16, name=f"ob{p}_{qb}_{hh}_{c}",
                                         tag="osb")
                        nc.vector.tensor_scalar_mul(osb, po[hh][:, c * 65:c * 65 + 64],
                                                    rec)
                        nc.sync.dma_start(
                            out=out[qb * 512 + c * 128:qb * 512 + (c + 1) * 128,
                                    h * 64:(h + 1) * 64],
                            in_=osb)

            # Dependency tracking is emission-order based: every producer must
            # be emitted before its consumers. Scheduling PRIORITY, however, is
            # tc.cur_priority, which we can band-shift: V is emitted early (so
            # PV sees its writes) but in a low-priority band, making it PE
            # slack-filler behind the ACT-feeding scores stream.
            # Emission order: minimal prerequisites of attention (0, qb=0)
            # first -- one q block + all of kP[0] -- then V, then the rest.
            qk_half(0, 0, 0)            # qP[0] block qb=0
            qk_group(0, 2)              # kP[0] first half of S
            qk_group(1, 2)              # kP[0] second half of S
            p_save = tc.cur_priority
            tc.cur_priority = p_save + 600
            qk_half(0, 0, 1)            # qP[0] qb=1 -- unblocks iter (0,1)
            tc.cur_priority = p_save + 800
            for st in range(NST):
                v_group(st)
            tc.cur_priority = p_save + 1600
            qk_group(1, 0)              # qP[0] qb=2,3
            qk_group(0, 3)              # kP[1]
            qk_group(1, 3)
            qk_group(0, 1)              # qP[1]
            qk_group(1, 1)
            p_proj_end = tc.cur_priority
            tc.cur_priority = p_save
            for qb in range(NQB):
                attention_iter(0, qb)
            tc.cur_priority = max(tc.cur_priority, p_proj_end)
            for qb in range(NQB):
                attention_iter(1, qb)
    return nc


def _relax_waits(nc):
    """Walrus rejects instructions carrying more than ~1 embedded semaphore
    wait ("Too many sync wait commands"). Strip waits that are provably
    redundant. Soundness (this kernel is fully unrolled: no loops, no sem
    resets, all sems monotone):
      R1: a PE instruction never needs a wait on PE's own completion
          semaphore: PE executes in order, never reads its own output
          (no PSUM read port), and drains (PSUM writes) are in order.
      R2: a wait (sem >= v) is redundant if an earlier instruction on the
          same engine already waits (sem >= v' >= v): the per-engine
          sequencer processes waits in stream order.
    Returns the number of instructions still carrying >1 ge-waits."""
    # Only PE: it never reads its own writes (no PSUM read port), and its
    # in-order drain sequences PSUM WAW. DVE/ACT have deep non-interlocked
    # pipelines -- their self-waits guard real RAW hazards.
    own_sem = {"PE": "PE_"}
    observed = {}  # (engine, sem id) -> max value waited
    remaining = 0
    for fn in nc.m.functions:
        for blk in fn.blocks:
            for inst in blk.instructions:
                si = getattr(inst, "sync_info", None)
                if si is None or not si.on_wait:
                    continue
                eng = str(inst.engine).split(".")[-1]
                pfx = own_sem.get(eng)
                keep, nge = [], 0
                for w in si.on_wait:
                    if w.sync_type != "semaphore" or w.wait_mode != "sem-ge-imm" \
                            or w.wait_reg is not None \
                            or w.ant_name.startswith("barrier_"):
                        # barrier sems are decremented (non-monotone): hands off
                        keep.append(w)
                        continue
                    if pfx is not None and w.ant_name.startswith(pfx):
                        continue  # R1
                    k = (eng, w.id)
                    if observed.get(k, -1) >= w.wait_value:
                        continue  # R2
                    observed[k] = w.wait_value
                    keep.append(w)
                    nge += 1
                if nge > 1:
                    remaining += 1
                if len(keep) != len(si.on_wait):
                    si.on_wait = keep
                    inst.sync_info = si
    return remaining


def _split_multi_waits(nc):
    """Any instruction still carrying >1 ge-waits after relaxation gets its
    excess waits hoisted onto same-engine NoOps inserted right before it
    (a sequence of single-wait instructions is semantically identical to one
    multi-wait instruction on an in-order sequencer)."""
    import bass_rust

    def wkey(w):
        return (w.id, w.wait_value, w.wait_mode)

    plan = {}
    for fn in nc.m.functions:
        for blk in fn.blocks:
            for inst in blk.instructions:
                si = getattr(inst, "sync_info", None)
                if si is None or not si.on_wait:
                    continue
                ow = list(si.on_wait)
                ge = [w for w in ow
                      if w.sync_type == "semaphore" and w.wait_mode == "sem-ge-imm"
                      and w.wait_reg is None
                      and not w.ant_name.startswith("barrier_")]
                if len(ge) <= 1:
                    continue
                hoist = ge[1:]
                hkeys = {wkey(w) for w in hoist}
                nops = []
                for w in hoist:
                    nb = nc.engines[inst.engine].nop(nofuse=True, hint="wait_split")
                    ni = nb.ins
                    ni.sync_info = bass_rust.SyncInfo(on_wait=[w], on_update=[])
                    nops.append(ni)
                plan[inst.name] = nops
                si.on_wait = [w for w in ow if wkey(w) not in hkeys
                              or (w.sync_type, w.wait_mode) != ("semaphore", "sem-ge-imm")]
                inst.sync_info = si
    if not plan:
        return 0
    created = {n.name for nops in plan.values() for n in nops}
    for fn in nc.m.functions:
        for blk in fn.blocks:
            cur = list(blk.instructions)
            new = []
            for i in cur:
                if i.name in created:
                    continue
                if i.name in plan:
                    new.extend(plan[i.name])
                new.append(i)
            blk.instructions = new
    return len(plan)


def get_nc():
    if "nc" not in _CACHE:
        nc = _build_nc()
        _relax_waits(nc)
        _split_multi_waits(nc)
        _CACHE["nc"] = nc
    return _CACHE["nc"]


def prep_inputs(x, W_qkv, b_qkv):
    """Host-side sharding: returns the 8 per-core input maps."""
    x = np.asarray(x, dtype=np.float32)
    W_qkv = np.asarray(W_qkv, dtype=np.float32)
    b_qkv = np.asarray(b_qkv, dtype=np.float32)
    ones = np.ones((1, S), np.float32)
    in_maps = []
    for c in range(8):
        b, g = divmod(c, 4)
        xTm = np.concatenate([np.ascontiguousarray(x[b].T), ones], axis=0).astype(np.float16)
        heads = list(range(HL * g, HL * g + HL))
        cols = np.concatenate([np.arange(h * Dh, (h + 1) * Dh) for h in heads])
        wqk_m = np.empty((D, 512), np.float16)
        wqk_m[:, :256] = W_qkv[:, cols]
        wqk_m[:, 256:] = W_qkv[:, D + cols]
        # bias columns: [q pair0 | q pair1 | k pair0 | k pair1]
        bqk_m = np.empty((128, 4), np.float32)
        bqk_m[:, 0] = b_qkv[cols[:128]]
        bqk_m[:, 1] = b_qkv[cols[128:]]
        bqk_m[:, 2] = b_qkv[D + cols[:128]]
        bqk_m[:, 3] = b_qkv[D + cols[128:]]
        wv_m = np.zeros((KC, VW), np.float16)
        for i, h in enumerate(heads):
            vcols = 2 * D + h * Dh
            wv_m[:D, i * 65:i * 65 + 64] = W_qkv[:, vcols:vcols + Dh]
            wv_m[D, i * 65:i * 65 + 64] = b_qkv[vcols:vcols + Dh]
            wv_m[D, i * 65 + 64] = 1.0  # generates the denominator column
        in_maps.append({"xT": xTm, "wqk": wqk_m, "wv": wv_m, "bqk": bqk_m})
    return in_maps


def assemble_output(results):
    out = np.empty((B, S, D), np.float32)
    for c in range(8):
        b, g = divmod(c, 4)
        out[b, :, g * GW:(g + 1) * GW] = np.asarray(results[c]["out"]).astype(np.float32)
    return out


def kernel(x, W_qkv, b_qkv):
    from concourse.bass_utils import run_bass_kernel_spmd

    nc = get_nc()
    in_maps = prep_inputs(x, W_qkv, b_qkv)
    res = run_bass_kernel_spmd(nc, in_maps, list(range(8)))
    return assemble_output(res.results)


# revision 24
# speedup vs baseline: 1.0910x; 1.0080x over previous
"""Fused QKV-projection + multi-head attention kernel for Trainium2.

Problem: x [2, 2048, 1024] fp32; W_qkv [1024, 3072]; b_qkv [3072].
  qkv = x @ W_qkv + b; 16 heads of 64; scores = q k^T / 8; softmax; out = attn @ v.

Sharding: 8 cores = 2 (batch) x 4 (head groups of 4). Each core is fully
independent (no collectives): projection for its batch restricted to its 4
heads' q/k/v columns, then attention for those heads.

Per-core design:
  - host feeds x^T with an appended ones-row (feeds V's bias fold);
    q/k biases are applied on DVE during the PSUM->SBUF copy
    (tensor_scalar_add with a per-partition [128,1] bias column), so the
    q/k projection contracts over exactly 1024 rows = 8 full PE chunks.
  - q/k are produced TRANSPOSED and packed in head-PAIR tiles [128, S]
    (rows 0-63 = even head, 64-127 = odd head). The two halves drive two
    matmuls on disjoint PE row-groups (tile_position auto-derived from the
    base partition) that execute CONCURRENTLY on the 128x128 array --
    recovering the half-array loss of the Dh=64 contraction.
  - scores^T = kT.T @ qT needs no transposes anywhere; softmax is a single
    fused exp on ScalarE (scale=1/8 applied by the ACT datapath; no max
    subtraction -- scores are O(+-8), well within fp16/fp32 exp range).
  - V tiles are [128 k, 4 heads x 65]: per head 64 projected v-cols plus a
    ones column generated by the bias row (x's appended ones-row), which
    rides the PV matmul to produce the softmax denominators.
  - PV runs FLIPPED: out[q, v] = expS.T @ V_aug with the EXP TILE as the
    matmul stationary ([128 k, 128 q] f16, 128 cols => compiler-automatic
    fast weight load; measured 40.8 ns/matmul steady state) and V_aug
    [128 k, 65] as the stream: 65 streamed columns per 128 q-positions,
    ~2.3x fewer PE stream cycles than streaming exp past a V stationary.
    Each (head, q-chunk) accumulator chain closes its PSUM group before
    the next opens in that bank (one pending group per psum zero-region).
    The denominator lands as a per-PARTITION column, so normalization is a
    [128,1] reciprocal + per-partition-scalar multiply -- sub-microsecond,
    with no single-partition row ops, no DRAM bounce, and a tiny tail.
    The exp is pre-scaled by e^-9 (free ACT bias, cancels exactly in the
    softmax ratio) so denominators sit comfortably in fp16.
  - the output is stored in natural [seq, head-dim] orientation, fp16
    (host upcasts).
  - matmul operands are fp16 (measured end-to-end rel err ~1.4e-3 vs the fp32
    reference; strict-fp32 matmuls are 4x slower, fp32r trips walrus sync
    limits). PSUM accumulation is fp32.

Scheduling: Tile tracks dependencies in EMISSION order, while scheduler
priority is tc.cur_priority -- V production is emitted early (correct deps)
but in a low-priority band so it fills PE slack behind the ACT-bound exp
stream; the pair-1 projection overlaps pair-0's attention the same way.
Emission order is arranged so pair-0 attention's prerequisites (one q
block + all of kP[0] + V) are produced first.

Sync: this walrus build rejects instructions carrying more than one embedded
semaphore wait. _relax_waits() strips provably redundant waits (PE self-waits;
same-engine-covered waits) and _split_multi_waits() hoists any remaining
excess onto single-wait NoOps inserted before the instruction.
"""

import sys

if "/opt/trn_rl_repo" not in sys.path:
    sys.path.insert(0, "/opt/trn_rl_repo")

import numpy as np
from contextlib import ExitStack

B, S, D = 2, 2048, 1024
H, Dh = 16, 64
HL = 4          # heads per core
GW = HL * Dh    # 256 output cols per core
VW = HL * 65    # V width: per head [v (64) | ones (1)] -- ones col via bias row
KC = 1025       # augmented contraction for V (1024 + bias row)
NST = S // 128  # 16 s-tiles
NQB = S // 512  # 4 q blocks

_CACHE = {}


def _build_nc():
    import concourse.bass as bass
    import concourse.mybir as mybir
    import concourse.tile as tile

    f32 = mybir.dt.float32
    f16 = mybir.dt.float16
    bf16 = mybir.dt.bfloat16
    Exp = mybir.ActivationFunctionType.Exp

    nc = bass.Bass()
    xT = nc.dram_tensor("xT", [KC, S], f16, kind="ExternalInput")
    wqk = nc.dram_tensor("wqk", [D, 512], f16, kind="ExternalInput")
    wv = nc.dram_tensor("wv", [KC, VW], f16, kind="ExternalInput")
    bqk = nc.dram_tensor("bqk", [128, 4], f32, kind="ExternalInput")
    # output is stored in natural [seq, head-dim] orientation, fp16 (host
    # upcasts): the PV matmuls run FLIPPED -- stationary = exp tile slice
    # [128 k, 128 q] (f16, 128-col => compiler-automatic fast weight load:
    # measured 40.8 ns/matmul steady state), stream = V_aug [128 k, 65]
    # (64 v-cols + ones col). 65 streamed columns per 128 q-positions is
    # ~2.3x fewer PE stream cycles than the [v,q] orientation, and the
    # softmax denominator lands as a per-PARTITION column [128,1], making
    # normalization two tiny DVE ops (reciprocal [128,1] + tensor_scalar
    # multiply). The exp is emitted pre-scaled by e^-9 (free ACT bias,
    # cancels exactly in the softmax ratio) so denominators sit in fp16
    # range.
    out = nc.dram_tensor("out", [S, GW], f16, kind="ExternalOutput")

    def chunks(n=8):
        for d in range(n):
            yield d, 128

    def vchunks():
        for d in range(9):
            yield d, (128 if d < 8 else 1)

    with tile.TileContext(nc) as tc, ExitStack() as ctx:
        persist = ctx.enter_context(tc.tile_pool(name="persist", bufs=1))
        # q/k head-PAIR tiles: rows 0..63 = head 2p, 64..127 = head 2p+1.
        qP = [persist.tile([128, S], f16, name=f"qP{p}", tag=f"qP{p}") for p in range(2)]
        kP = [persist.tile([128, S], f16, name=f"kP{p}", tag=f"kP{p}") for p in range(2)]
        V = [persist.tile([128, VW], f16, name=f"V{t}", tag=f"V{t}") for t in range(NST)]
        bqk_sb = persist.tile([128, 4], f32, name="bqk_sb", tag="bqk_sb")
        # exp pre-shift constant (see `out` comment): exp(s/8 - 9)
        nbias = persist.tile([128, 1], f32, name="nbias", tag="nbias")
        nc.vector.memset(nbias, -9.0)

        wpool = ctx.enter_context(tc.tile_pool(name="wpool", bufs=1))
        xpool = ctx.enter_context(tc.tile_pool(name="xpool", bufs=1))

        # input DMAs, ordered so pair-0 attention prerequisites land first.
        # Weights and x are split into SEPARATE tiles at the granularity the
        # first projection chain consumes (per-tile dependency tracking), so
        # the first matmul waits on ~160KB, not the whole input set:
        #   wqA [128,128] = q-pair-0 weight block, wqB [128,384] = the rest;
        #   xA/xB = 512-col halves of each 1024-col x chunk.
        nc.sync.dma_start(out=bqk_sb, in_=bqk[:, :])
        wqA, wqB, wv_sb = [], [], []
        xA = [[], []]
        xB = [[], []]
        for d, p in chunks():
            twq = wpool.tile([p, 128], f16, name=f"wqA{d}", tag=f"wqA{d}")
            nc.sync.dma_start(out=twq, in_=wqk[d * 128:d * 128 + p, 0:128])
            wqA.append(twq)
            t0 = xpool.tile([p, 512], f16, name=f"xA0_{d}", tag=f"xA0_{d}")
            nc.sync.dma_start(out=t0, in_=xT[d * 128:d * 128 + p, 0:512])
            xA[0].append(t0)
        for d, p in chunks():
            twq = wpool.tile([p, 384], f16, name=f"wqB{d}", tag=f"wqB{d}")
            nc.sync.dma_start(out=twq, in_=wqk[d * 128:d * 128 + p, 128:512])
            wqB.append(twq)
            t0 = xpool.tile([p, 512], f16, name=f"xB0_{d}", tag=f"xB0_{d}")
            nc.sync.dma_start(out=t0, in_=xT[d * 128:d * 128 + p, 512:1024])
            xB[0].append(t0)
        for d, p in chunks():
            t1 = xpool.tile([p, 512], f16, name=f"xA1_{d}", tag=f"xA1_{d}")
            nc.sync.dma_start(out=t1, in_=xT[d * 128:d * 128 + p, 1024:1536])
            xA[1].append(t1)
            t1 = xpool.tile([p, 512], f16, name=f"xB1_{d}", tag=f"xB1_{d}")
            nc.sync.dma_start(out=t1, in_=xT[d * 128:d * 128 + p, 1536:2048])
            xB[1].append(t1)
        # V inputs: 9th chunk of x (the ones row, feeds V's bias fold)
        x8 = []
        for d, p in vchunks():
            twv = wpool.tile([p, VW], f16, name=f"wv{d}", tag=f"wv{d}")
            nc.sync.dma_start(out=twv, in_=wv[d * 128:d * 128 + p, :])
            wv_sb.append(twv)
            if d == 8:
                for sh in range(2):
                    t8 = xpool.tile([p, 1024], f16, name=f"x8_{sh}", tag=f"x8_{sh}")
                    nc.sync.dma_start(
                        out=t8, in_=xT[1024:1025, sh * 1024:(sh + 1) * 1024])
                    x8.append(t8)

        def xstream(sh, j, d):
            """x operand [*,512] for q-block j of seq-half sh, chunk d."""
            return (xA if j == 0 else xB)[sh][d]

        def wq_block(mt, d):
            """weight stationary [*,128] for projection M-tile mt, chunk d."""
            return wqA[d] if mt == 0 else wqB[d][:, (mt - 1) * 128:mt * 128]

        with tc.tile_pool(name="psA", bufs=2, space="PSUM") as psA, \
             tc.tile_pool(name="expp", bufs=40) as expp, \
             tc.tile_pool(name="normp", bufs=6) as normp, \
             tc.tile_pool(name="psS", bufs=2, space="PSUM") as psS, \
             tc.tile_pool(name="psO", bufs=2, space="PSUM") as psO:

            def qk_half(sh, mt, j):
                """Half a projection M-tile (one 512-col q-block) -> qP/kP.
                mt 0/1 -> q pairs 0/1, mt 2/3 -> k pairs 0/1. Bias applied
                on DVE during the PSUM->SBUF copy (bqk col: q pairs 0/1 ->
                cols 0/1, k pairs 0/1 -> cols 2/3)."""
                dst = (qP if mt < 2 else kP)[mt % 2]
                ps = psA.tile([128, 512], f32, name=f"psA{sh}_{mt}_{j}", tag="psA")
                for d, p in chunks():
                    nc.tensor.matmul(ps, wq_block(mt, d), xstream(sh, j, d),
                                     start=(d == 0), stop=(d == 7))
                qb = sh * 2 + j
                bcol = (mt % 2) if mt < 2 else (2 + mt % 2)
                nc.vector.tensor_scalar_add(
                    dst[:, qb * 512:(qb + 1) * 512], ps, bqk_sb[:, bcol:bcol + 1])

            def qk_group(sh, mt):
                qk_half(sh, mt, 0)
                qk_half(sh, mt, 1)

            def v_group(st):
                sh, stl = divmod(st, 8)
                psv = psA.tile([128, VW], f32, name=f"psV{st}", tag="psA")
                for d, p in vchunks():
                    if d < 8:
                        xop = (xA if stl < 4 else xB)[sh][d][
                            :, (stl % 4) * 128:(stl % 4) * 128 + 128]
                    else:
                        xop = x8[sh][:, stl * 128:(stl + 1) * 128]
                    nc.tensor.matmul(psv, xop, wv_sb[d],
                                     start=(d == 0), stop=(d == 8))
                nc.vector.tensor_copy(V[st], psv)

            def attention_iter(p, qb):
                # packed scores^T: head 2p on PE rows 0-63 -> psS bank 0,
                # head 2p+1 on rows 64-127 -> bank 1; one fused exp over both
                ets = []
                for st in range(NST):
                    ps = psS.tile([128, 1024], f32, name=f"s{p}_{qb}_{st}", tag="psS")
                    for hh in range(2):
                        r0, r1 = hh * 64, hh * 64 + 64
                        nc.tensor.matmul(
                            ps[:, hh * 512:(hh + 1) * 512],
                            kP[p][r0:r1, st * 128:(st + 1) * 128],
                            qP[p][r0:r1, qb * 512:(qb + 1) * 512],
                            start=True, stop=True)
                    et = expp.tile([128, 1024], f16, name=f"e{p}_{qb}_{st}", tag="expS")
                    nc.scalar.activation(et, ps, Exp, scale=0.125, bias=nbias[:, 0:1])
                    ets.append(et)

                # FLIPPED PV: out[q, v] = exp(S^T).T @ V_aug, accumulated
                # over st. Stationary = exp slice [128 k, 128 q] (fast
                # weight load), stream = V_aug [128 k, 65]. Each head gets
                # 4 q-chunk accumulator chains packed at 65-col offsets in
                # one PSUM tile; col 64 of each chunk = the softmax
                # denominator as a per-partition column.
                po = [psO.tile([128, 260], f32, name=f"po{p}_{qb}_{hh}",
                               tag="psO") for hh in range(2)]
                # chain-major: each (head, q-chunk) chain closes its psum
                # accumulation group before the next opens in that bank
                # (one pending group per psum zero-region). PV trails the
                # exp stream by up to one iteration; expp is deep enough.
                # PV runs in a band below scores (the next iteration's
                # scores must preempt it -- ACT is the pacer and otherwise
                # starves ~5us at every iteration boundary) but above the
                # late projection band (expp must keep draining).
                pv_pri = tc.cur_priority
                tc.cur_priority = pv_pri + 1200
                for hh in range(2):
                    h = 2 * p + hh
                    for c in range(4):
                        for st in range(NST):
                            nc.tensor.matmul(
                                po[hh][:, c * 65:(c + 1) * 65],
                                ets[st][:, hh * 512 + c * 128:hh * 512 + (c + 1) * 128],
                                V[st][:, h * 65:(h + 1) * 65],
                                start=(st == 0), stop=(st == NST - 1))
                tc.cur_priority = pv_pri
                # normalize: per (head, q-chunk) a [128,1] reciprocal and a
                # per-partition-scalar multiply -- tiny full-width DVE ops
                for hh in range(2):
                    h = 2 * p + hh
                    for c in range(4):
                        rec = normp.tile([128, 1], f32, name=f"rc{p}_{qb}_{hh}_{c}",
                                         tag="rec")
                        nc.vector.reciprocal(rec, po[hh][:, c * 65 + 64:c * 65 + 65])
                        osb = normp.tile([128, 64], f16, name=f"ob{p}_{qb}_{hh}_{c}",
                                         tag="osb")
                        nc.vector.tensor_scalar_mul(osb, po[hh][:, c * 65:c * 65 + 64],
                                                    rec)
                        nc.sync.dma_start(
                            out=out[qb * 512 + c * 128:qb * 512 + (c + 1) * 128,
                                    h * 64:(h + 1) * 64],
                            in_=osb)

            # Dependency tracking is emission-order based: every producer must
            # be emitted before its consumers. Scheduling PRIORITY, however, is
            # tc.cur_priority, which we can band-shift: V is emitted early (so
            # PV sees its writes) but in a low-priority band, making it PE
            # slack-filler behind the ACT-feeding scores stream.
            # Emission order: minimal prerequisites of attention (0, qb=0)
            # first -- one q block + all of kP[0] -- then V, then the rest.
            qk_half(0, 0, 0)            # qP[0] block qb=0
            qk_group(0, 2)              # kP[0] first half of S
            qk_group(1, 2)              # kP[0] second half of S
            p_save = tc.cur_priority
            tc.cur_priority = p_save + 600
            qk_half(0, 0, 1)            # qP[0] qb=1 -- unblocks iter (0,1)
            qk_group(1, 0)              # qP[0] qb=2,3 -- unblocks iters (0,2/3)
            tc.cur_priority = p_save + 800
            for st in range(NST):
                v_group(st)
            tc.cur_priority = p_save + 1600
            qk_group(0, 3)              # kP[1]
            qk_group(1, 3)
            qk_group(0, 1)              # qP[1]
            qk_group(1, 1)
            p_proj_end = tc.cur_priority
            tc.cur_priority = p_save
            for qb in range(NQB):
                attention_iter(0, qb)
            tc.cur_priority = max(tc.cur_priority, p_proj_end)
            for qb in range(NQB):
                attention_iter(1, qb)
    return nc


def _relax_waits(nc):
    """Walrus rejects instructions carrying more than ~1 embedded semaphore
    wait ("Too many sync wait commands"). Strip waits that are provably
    redundant. Soundness (this kernel is fully unrolled: no loops, no sem
    resets, all sems monotone):
      R1: a PE instruction never needs a wait on PE's own completion
          semaphore: PE executes in order, never reads its own output
          (no PSUM read port), and drains (PSUM writes) are in order.
      R2: a wait (sem >= v) is redundant if an earlier instruction on the
          same engine already waits (sem >= v' >= v): the per-engine
          sequencer processes waits in stream order.
    Returns the number of instructions still carrying >1 ge-waits."""
    # Only PE: it never reads its own writes (no PSUM read port), and its
    # in-order drain sequences PSUM WAW. DVE/ACT have deep non-interlocked
    # pipelines -- their self-waits guard real RAW hazards.
    own_sem = {"PE": "PE_"}
    observed = {}  # (engine, sem id) -> max value waited
    remaining = 0
    for fn in nc.m.functions:
        for blk in fn.blocks:
            for inst in blk.instructions:
                si = getattr(inst, "sync_info", None)
                if si is None or not si.on_wait:
                    continue
                eng = str(inst.engine).split(".")[-1]
                pfx = own_sem.get(eng)
                keep, nge = [], 0
                for w in si.on_wait:
                    if w.sync_type != "semaphore" or w.wait_mode != "sem-ge-imm" \
                            or w.wait_reg is not None \
                            or w.ant_name.startswith("barrier_"):
                        # barrier sems are decremented (non-monotone): hands off
                        keep.append(w)
                        continue
                    if pfx is not None and w.ant_name.startswith(pfx):
                        continue  # R1
                    k = (eng, w.id)
                    if observed.get(k, -1) >= w.wait_value:
                        continue  # R2
                    observed[k] = w.wait_value
                    keep.append(w)
                    nge += 1
                if nge > 1:
                    remaining += 1
                if len(keep) != len(si.on_wait):
                    si.on_wait = keep
                    inst.sync_info = si
    return remaining


def _split_multi_waits(nc):
    """Any instruction still carrying >1 ge-waits after relaxation gets its
    excess waits hoisted onto same-engine NoOps inserted right before it
    (a sequence of single-wait instructions is semantically identical to one
    multi-wait instruction on an in-order sequencer)."""
    import bass_rust

    def wkey(w):
        return (w.id, w.wait_value, w.wait_mode)

    plan = {}
    for fn in nc.m.functions:
        for blk in fn.blocks:
            for inst in blk.instructions:
                si = getattr(inst, "sync_info", None)
                if si is None or not si.on_wait:
                    continue
                ow = list(si.on_wait)
                ge = [w for w in ow
                      if w.sync_type == "semaphore" and w.wait_mode == "sem-ge-imm"
                      and w.wait_reg is None
                      and not w.ant_name.startswith("barrier_")]
                if len(ge) <= 1:
                    continue
                hoist = ge[1:]
                hkeys = {wkey(w) for w in hoist}
                nops = []
                for w in hoist:
                    nb = nc.engines[inst.engine].nop(nofuse=True, hint="wait_split")
                    ni = nb.ins
                    ni.sync_info = bass_rust.SyncInfo(on_wait=[w], on_update=[])
                    nops.append(ni)
                plan[inst.name] = nops
                si.on_wait = [w for w in ow if wkey(w) not in hkeys
                              or (w.sync_type, w.wait_mode) != ("semaphore", "sem-ge-imm")]
                inst.sync_info = si
    if not plan:
        return 0
    created = {n.name for nops in plan.values() for n in nops}
    for fn in nc.m.functions:
        for blk in fn.blocks:
            cur = list(blk.instructions)
            new = []
            for i in cur:
                if i.name in created:
                    continue
                if i.name in plan:
                    new.extend(plan[i.name])
                new.append(i)
            blk.instructions = new
    return len(plan)


def get_nc():
    if "nc" not in _CACHE:
        nc = _build_nc()
        _relax_waits(nc)
        _split_multi_waits(nc)
        _CACHE["nc"] = nc
    return _CACHE["nc"]


def prep_inputs(x, W_qkv, b_qkv):
    """Host-side sharding: returns the 8 per-core input maps."""
    x = np.asarray(x, dtype=np.float32)
    W_qkv = np.asarray(W_qkv, dtype=np.float32)
    b_qkv = np.asarray(b_qkv, dtype=np.float32)
    ones = np.ones((1, S), np.float32)
    in_maps = []
    for c in range(8):
        b, g = divmod(c, 4)
        xTm = np.concatenate([np.ascontiguousarray(x[b].T), ones], axis=0).astype(np.float16)
        heads = list(range(HL * g, HL * g + HL))
        cols = np.concatenate([np.arange(h * Dh, (h + 1) * Dh) for h in heads])
        wqk_m = np.empty((D, 512), np.float16)
        wqk_m[:, :256] = W_qkv[:, cols]
        wqk_m[:, 256:] = W_qkv[:, D + cols]
        # bias columns: [q pair0 | q pair1 | k pair0 | k pair1]
        bqk_m = np.empty((128, 4), np.float32)
        bqk_m[:, 0] = b_qkv[cols[:128]]
        bqk_m[:, 1] = b_qkv[cols[128:]]
        bqk_m[:, 2] = b_qkv[D + cols[:128]]
        bqk_m[:, 3] = b_qkv[D + cols[128:]]
        wv_m = np.zeros((KC, VW), np.float16)
        for i, h in enumerate(heads):
            vcols = 2 * D + h * Dh
            wv_m[:D, i * 65:i * 65 + 64] = W_qkv[:, vcols:vcols + Dh]
            wv_m[D, i * 65:i * 65 + 64] = b_qkv[vcols:vcols + Dh]
            wv_m[D, i * 65 + 64] = 1.0  # generates the denominator column
        in_maps.append({"xT": xTm, "wqk": wqk_m, "wv": wv_m, "bqk": bqk_m})
    return in_maps


def assemble_output(results):
    out = np.empty((B, S, D), np.float32)
    for c in range(8):
        b, g = divmod(c, 4)
        out[b, :, g * GW:(g + 1) * GW] = np.asarray(results[c]["out"]).astype(np.float32)
    return out


def kernel(x, W_qkv, b_qkv):
    from concourse.bass_utils import run_bass_kernel_spmd

    nc = get_nc()
    in_maps = prep_inputs(x, W_qkv, b_qkv)
    res = run_bass_kernel_spmd(nc, in_maps, list(range(8)))
    return assemble_output(res.results)
